# revision 50
# baseline (speedup 1.0000x reference)
"""Trainium2 Bass kernel for causal self-attention with doubled rotary.

Full-input contract: kernel(**inputs) takes the complete tensors
(x [4,2048,2048], wq/wk/wv/wo [2048,2048]) and returns [4,2048,2048] fp32.

Sharding: 8 cores = 4 batch elements x 2 head-halves (8 heads each).
Each core computes a partial output projection (its heads' columns of wo);
the host sums the two partials per batch element.

All matmul operands are fp16 (10 mantissa bits, ~fp32r accuracy for this
data): full-rate PE streaming at any width, fast weight loads (FWL), half
the SBUF/DMA footprint of fp32r.

Per-core structure (engine streams execute in emission order, so independent
work is interleaved at emission time to keep the PE dense):
  - group g in 0..3 owns heads (2g, 2g+1): q/k/v projections stream 512-wide
    x panels against 128-col weight stationaries; doubled-angle rotary
    (R(t)^2 == R(2t)) on DVE in fp16; all into double-buffered SBUF tiles.
  - attention pair g-1 is emitted interleaved with the projection of group
    g: QK^T computed transposed (ST[s,t]) so exp(ST) feeds the PV matmul
    directly with v as the stationary operand. Causal diagonal chunks are
    sliced at the 128-column grid; the triangular mask is applied as a
    cheap fp16 multiply on the exp output (zeroing masked lanes exactly).
    Softmax denominators: per-partition partial sums accumulate on the DVE
    in fp16 (fast 2-byte all-SBUF mode); one 512-wide all-ones matmul per
    panel does the cross-partition reduce + broadcast; full-width
    reciprocal; normalization rides the PSUM->SBUF copy of the PV output,
    written straight into a resident fp16 y buffer (no DRAM spill).
  - the output projection reads y and a fully resident wo directly from
    SBUF; panel jp is emitted interleaved with attention panel jp+1 of the
    last pair, with three (jp, hl) attention streams run abreast at the
    phase seam to hide the QK->exp->PV latency.

DMA discipline (per-queue transfer bandwidth is the scarce resource):
inputs are host-pre-swizzled so every transfer is a few fat contiguous
runs per partition; x low/high halves ride the SP/ACT HWDGE queues, w on
ACT (wq double-buffered, prefetched a group ahead), the Pool SWDGE queue
is kept clear for the latency-critical rotary half-swap copies, and the
wo preload is chunked between x panels on SP.
"""

import os
import sys

for _p in ("/opt/trn_rl_repo", "/root/.axon_site/_ro/trn_rl_repo"):
    if os.path.isdir(_p) and _p not in sys.path:
        sys.path.insert(0, _p)

import numpy as np

import concourse.bass as bass
import concourse.mybir as mybir
from concourse import bacc
from concourse.bass import ds
from concourse.tile import TileContext
from concourse.bass_utils import run_bass_kernel_spmd

F32 = mybir.dt.float32
F16 = mybir.dt.float16

P = 128          # partitions / head dim
T = 2048         # sequence length
E = 2048         # embedding dim
B = 4
HPC = 8          # heads per core
D = 128          # head dim
PAN = 512        # attention t-panel width (PSUM bank limit for fp32)
NPAN = T // PAN  # 4
XB = 512         # projection x-panel width
NXB = T // XB    # 4
EO = E // P      # 16 contraction chunks for projections
NGRP = 4         # head pairs per core
NCH = T // P     # 16 s-chunks (also v t-tiles)
SCALE = 1.0 / float(np.sqrt(D))

ADD = mybir.AluOpType.add
MULT = mybir.AluOpType.mult
EXP = mybir.ActivationFunctionType.Exp


def _zip_emit(*lists):
    """Emit thunks from several lists round-robin, proportionally."""
    lists = [list(l) for l in lists if l]
    if not lists:
        return
    total = max(len(l) for l in lists)
    idx = [0.0] * len(lists)
    step = [len(l) / total for l in lists]
    for _ in range(total):
        for li, l in enumerate(lists):
            idx[li] += step[li]
            while idx[li] >= 1.0 and l:
                l.pop(0)()
                idx[li] -= 1.0
    for l in lists:
        for f in l:
            f()


class Ctx:
    pass


def build_program():
    nc = bacc.Bacc()
    cx = Ctx()
    cx.nc = nc

    # Pre-swizzled on the host so every DMA is a few fat contiguous runs
    # per partition (descriptor-count, not bandwidth, limits DMA here).
    cx.xP = nc.declare_dram_parameter("xP", [P, NXB, EO, XB], F16, isOutput=False)
    cx.wqP = nc.declare_dram_parameter("wqP", [P, NGRP, EO, 2 * D], F16, isOutput=False)
    cx.wkP = nc.declare_dram_parameter("wkP", [P, NGRP, EO, 2 * D], F16, isOutput=False)
    cx.wvP = nc.declare_dram_parameter("wvP", [P, NGRP, EO, 2 * D], F16, isOutput=False)
    cx.woP = nc.declare_dram_parameter("woP", [P, HPC, E], F16, isOutput=False)
    cx.cos2 = nc.declare_dram_parameter("cos2", [P, T], F16, isOutput=False)
    cx.sin2 = nc.declare_dram_parameter("sin2", [P, T], F16, isOutput=False)
    cx.mask = nc.declare_dram_parameter("mask", [P, P], F16, isOutput=False)
    cx.out = nc.declare_dram_parameter("out", [E, T], F32, isOutput=True)

    with TileContext(nc) as tc:
        cx.tc = tc
        with (
            tc.tile_pool(name="const", bufs=1) as cpool,
            tc.tile_pool(name="yt", bufs=1) as ytpool,
            tc.tile_pool(name="wo", bufs=1) as wopool,
        ):
            # Small resident tensors go first on the Pool SWDGE queue; the
            # queue is then reserved for the latency-sensitive rotary swaps
            # (proj phases) and output stores (outproj phase). The big wo
            # preload is chunked onto the SP queue one piece per group.
            cx.c2 = cpool.tile([P, T], F16, tag="c2")
            nc.gpsimd.dma_start(cx.c2, cx.cos2[:, :])
            cx.s2 = cpool.tile([P, T], F16, tag="s2")
            nc.gpsimd.dma_start(cx.s2, cx.sin2[:, :])
            cx.mk = cpool.tile([P, P], F16, tag="mk")
            nc.gpsimd.dma_start(cx.mk, cx.mask[:, :])
            cx.onesmat = cpool.tile([P, P], F16, tag="onesmat")
            nc.vector.memset(cx.onesmat, 1.0)

            cx.wo_sb = wopool.tile([P, HPC, E], F16, tag="wo")

            def load_wo_chunk(c):
                def f():
                    nc.sync.dma_start(
                        cx.wo_sb[:, 2 * c : 2 * c + 2, :],
                        cx.woP[:, 2 * c : 2 * c + 2, :],
                    )
                return f
            cx.load_wo_chunk = load_wo_chunk

            # y^T for all 8 heads, written by attention finalize, read by
            # the output projection. One tile per pair keeps deps natural.
            cx.yt = []
            for g in range(NGRP):
                ytile = ytpool.tile([P, 2, T], F16, tag=f"yt{g}")
                cx.yt.append(ytile)

            with (
                tc.tile_pool(name="ex", bufs=4) as expool,
                tc.tile_pool(name="dn", bufs=2) as dnpool,
                tc.tile_pool(name="dsm", bufs=3) as dspool,
                tc.tile_pool(name="psS", bufs=3, space="PSUM") as psS,
                tc.tile_pool(name="psY", bufs=3, space="PSUM") as psY,
                tc.tile_pool(name="qk", bufs=2) as qkpool,
                tc.tile_pool(name="vp", bufs=2) as vpool,
            ):
                cx.expool, cx.dnpool, cx.dspool = expool, dnpool, dspool
                cx.psS, cx.psY = psS, psY
                cx.qkpool, cx.vpool = qkpool, vpool
                cx.qkv = {}  # g -> (qT, kT, v_sb)

                with (
                    tc.tile_pool(name="xp", bufs=2) as xpool,
                    tc.tile_pool(name="wq", bufs=2) as wqpool,
                    tc.tile_pool(name="wp", bufs=1) as wpool,
                    tc.tile_pool(name="rot", bufs=2) as rotpool,
                    tc.tile_pool(name="sw", bufs=2) as swpool,
                    tc.tile_pool(name="psP", bufs=2, space="PSUM") as psP,
                ):
                    cx.xpool, cx.wqpool, cx.wpool = xpool, wqpool, wpool
                    cx.rotpool, cx.swpool, cx.psP = rotpool, swpool, psP

                    for f in _proj_thunks(cx, 0):
                        f()
                    for g in range(1, NGRP):
                        proj = _proj_thunks(cx, g)
                        # the wo chunks ride the SP queue after x panel 1 of
                        # groups 1..3 (two in the last zip) so they never
                        # delay the x feed or the kernel start
                        proj.insert(19, cx.load_wo_chunk(g - 1))
                        if g == NGRP - 1:
                            proj.insert(40, cx.load_wo_chunk(3))
                        attn = _attn_thunks(cx, g - 1)
                        # start_group first (fires the w prefetch DMAs), then
                        # a head of attention chunks to keep the PE fed while
                        # the w transfers land, then proportional interleave.
                        proj[0]()
                        for f in attn[:20]:
                            f()
                        _zip_emit(proj[1:], attn[20:])

                with (
                    tc.tile_pool(name="ob", bufs=3) as opool,
                    tc.tile_pool(name="psO", bufs=2, space="PSUM") as psO,
                ):
                    cx.opool, cx.psO = opool, psO
                    # outproj(jp) may only be emitted after pair-3 has
                    # finalized panel jp: interleave panel jp's outproj with
                    # panel jp+1's attention chunks.
                    panels = [_attn_thunks(cx, NGRP - 1, only_jp=jp)
                              for jp in range(NPAN)]
                    oproj = [_outproj_thunks(cx, jp) for jp in range(NPAN)]
                    for f in panels[0]:
                        f()
                    for jp in range(1, NPAN):
                        _zip_emit(panels[jp], oproj[jp - 1])
                    for f in oproj[NPAN - 1]:
                        f()

    nc.finalize()
    return nc


def _proj_thunks(cx, g):
    """Thunk list for group g's projections + rotary."""
    nc = cx.nc
    thunks = []

    state = {}

    def start_group():
        # weight loads ride the ACT HWDGE queue (shared with the x high
        # halves); wq is double-buffered so its transfer prefetches a whole
        # group ahead. wk/wv are triggered after panel 0 so the ACT queue
        # serves transfers in the order the PE consumes them.
        wq_sb = cx.wqpool.tile([P, EO, 2 * D], F16, tag="wq")
        nc.scalar.dma_start(wq_sb[:, 0:8, :], cx.wqP[:, g, 0:8, :])
        nc.scalar.dma_start(wq_sb[:, 8:16, :], cx.wqP[:, g, 8:16, :])
        # per-panel tiles keep the scheduler's dependencies panel-precise:
        # attention on panel jp must not wait for later panels' rotary
        qTs, kTs, vs = [], [], []
        for xb in range(NXB):
            qt1 = cx.qkpool.tile([P, 2, XB], F16, tag=f"qT{xb}")
            qTs.append(qt1)
            kt1 = cx.qkpool.tile([P, 2, XB], F16, tag=f"kT{xb}")
            kTs.append(kt1)
            vt1 = cx.vpool.tile([P, (XB // P) * 2 * D], F16, tag=f"v{xb}")
            vs.append(vt1)
        cx.qkv[g] = (qTs, kTs, vs)
        cx._w = [wq_sb, None, None]

    def load_wk():
        wk_sb = cx.wpool.tile([P, EO, 2 * D], F16, tag="wk")
        nc.scalar.dma_start(wk_sb, cx.wkP[:, g, :, :])
        cx._w[1] = wk_sb

    def load_wv():
        wv_sb = cx.wpool.tile([P, EO, 2 * D], F16, tag="wv")
        nc.scalar.dma_start(wv_sb, cx.wvP[:, g, :, :])
        cx._w[2] = wv_sb

    thunks.append(start_group)

    def load_panel(xb):
        def f():
            xp = cx.xpool.tile([P, EO, XB], F16, tag="xp")
            # split across the SP and ACT HWDGE queues: halves land in
            # parallel, and the first qk group can start on the low eo
            # chunks while the high half is still in flight
            if g == 0 and xb == 0:
                # entirely on the SP queue in quarters: the ACT queue is
                # busy delivering wq/wk/wv for the cold start
                for q4 in range(4):
                    nc.sync.dma_start(
                        xp[:, 4 * q4 : 4 * q4 + 4, :],
                        cx.xP[:, xb, 4 * q4 : 4 * q4 + 4, :],
                    )
            else:
                nc.sync.dma_start(xp[:, 0:8, :], cx.xP[:, xb, 0:8, :])
                nc.scalar.dma_start(xp[:, 8:16, :], cx.xP[:, xb, 8:16, :])
            state[xb] = xp
        return f

    def qk_group(xb, wi, hl):
        def f():
            xp = state[xb]
            w_sb = cx._w[wi]
            dst = cx.qkv[g][wi][xb]
            psq = cx.psP.tile([P, PAN], F32, tag="psP")
            for eo in range(EO):
                nc.tensor.matmul(
                    psq,
                    lhsT=w_sb[:, eo, ds(hl * D, D)],
                    rhs=xp[:, eo, :],
                    start=(eo == 0),
                    stop=(eo == EO - 1),
                )
            nc.vector.tensor_copy(dst[:, hl, :], psq)
        return f

    def v_group(xb, tp):
        # two t-chunks (2*tp, 2*tp+1) share one PSUM tile; one fused cast
        def f():
            xp = state[xb]
            wv_sb = cx._w[2]
            v_sb = cx.qkv[g][2][xb]
            ps = cx.psP.tile([P, PAN], F32, tag="psP")
            for th in range(2):
                tt = 2 * tp + th
                psv = ps[:, ds(th * 2 * D, 2 * D)]
                for eo in range(EO):
                    nc.tensor.matmul(
                        psv,
                        lhsT=xp[:, eo, ds(tt * P, P)],
                        rhs=wv_sb[:, eo, :],
                        start=(eo == 0),
                        stop=(eo == EO - 1),
                    )
            nc.vector.tensor_copy(v_sb[:, ds(2 * tp * 2 * D, 4 * D)], ps)
        return f

    swq = {}

    def rot_swap(src_i, hl, xb):
        def f():
            src = cx.qkv[g][src_i][xb]
            qsw = cx.swpool.tile([P, XB], F16, tag="qsw")
            nc.gpsimd.dma_start(qsw[0:64, :], src[64:128, hl, :])
            nc.gpsimd.dma_start(qsw[64:128, :], src[0:64, hl, :])
            swq[(src_i, hl, xb)] = qsw
        return f

    def rot_apply(src_i, hl, xb):
        def f():
            src = cx.qkv[g][src_i][xb]
            sl = ds(xb * XB, XB)
            qsw = swq.pop((src_i, hl, xb))
            tmp = cx.rotpool.tile([P, XB], F16, tag="rtmp")
            nc.vector.tensor_tensor(tmp, qsw[:, :], cx.s2[:, sl], op=MULT)
            nc.vector.tensor_tensor(
                src[:, hl, :], src[:, hl, :], cx.c2[:, sl], op=MULT
            )
            nc.vector.tensor_tensor(src[:, hl, :], src[:, hl, :], tmp, op=ADD)
        return f

    for xb in range(NXB):
        thunks.append(load_panel(xb))
        if xb == 0:
            thunks.append(load_wk)
            thunks.append(load_wv)
        # swap DMAs are emitted right after the panel they read; the DVE
        # rotary ops two qk-groups later, giving the SWDGE round trip two
        # matmul groups (~14us) of slack before the DVE queue needs it.
        pend = []
        for wi in range(2):
            for hl in range(2):
                thunks.append(qk_group(xb, wi, hl))
                thunks.append(rot_swap(wi, hl, xb))
                pend.append((wi, hl, xb))
                if len(pend) > 2:
                    thunks.append(rot_apply(*pend.pop(0)))
        thunks.append(v_group(xb, 0))
        thunks.append(rot_apply(*pend.pop(0)))
        thunks.append(v_group(xb, 1))
        thunks.append(rot_apply(*pend.pop(0)))
    return thunks


def _attn_thunks(cx, g, only_jp=None):
    """Thunk list for the attention of head pair g (heads 2g, 2g+1)."""
    nc = cx.nc
    thunks = []
    st8 = cx.__dict__.setdefault(f"_attn_state_{g}", {})

    def chunk(hl, jp, i):
        def f():
            qTs, kTs, vs = cx.qkv[g]
            nch = 4 * jp + 4
            if i == 0:
                ytp = cx.psY.tile([P, PAN], F32, tag="psY")
                dsum = cx.dspool.tile([P, PAN], F16, tag="dsum")
                st8[(hl, jp)] = (ytp, dsum)
            ytp, dsum = st8[(hl, jp)]
            di = i - 4 * jp
            off = P * di if di > 0 else 0
            w = PAN - off
            st = cx.psS.tile([P, PAN], F32, tag="psS")
            stw = st[:, off:PAN]
            nc.tensor.matmul(
                stw,
                lhsT=kTs[i // 4][:, hl, ds((i % 4) * P, P)],
                rhs=qTs[jp][:, hl, ds(off, w)],
                start=True,
                stop=True,
            )
            ex = cx.expool.tile([P, PAN], F16, tag="ex")
            exw = ex[:, off:PAN]
            nc.scalar.activation(exw, stw, EXP, scale=SCALE)
            if di >= 0:
                # causal mask: zero the upper triangle of the diagonal block
                nc.vector.tensor_tensor(
                    ex[:, off:off + P], ex[:, off:off + P], cx.mk, op=MULT
                )
            last = i == nch - 1
            nc.tensor.matmul(
                ytp[:, off:PAN],
                lhsT=vs[i // 4][:, ds((i % 4) * 2 * D + hl * D, D)],
                rhs=exw,
                start=(i == 0),
                stop=last,
            )
            # softmax denominator: per-partition partial sums accumulate on
            # the DVE in fp16 (all-SBUF 2-byte ops run in the fast DVE
            # mode); one 512-wide all-ones matmul per panel finishes the
            # cross-partition reduce + broadcast at finalize.
            if i == 0:
                nc.vector.tensor_copy(dsum, ex[:, :])
            else:
                nc.vector.tensor_tensor(
                    dsum[:, off:PAN], dsum[:, off:PAN], exw, op=ADD
                )
        return f

    def finalize(hl, jp):
        def f():
            ytp, dsum = st8.pop((hl, jp))
            dps = cx.psS.tile([P, PAN], F32, tag="psS")
            nc.tensor.matmul(dps, lhsT=cx.onesmat, rhs=dsum, start=True, stop=True)
            rdb = cx.dnpool.tile([P, PAN], F32, tag="rdb")
            nc.vector.reciprocal_approx_fast(out=rdb, in_=dps)
            nc.vector.tensor_tensor(
                cx.yt[g][:, hl, ds(jp * PAN, PAN)], ytp, rdb, op=MULT
            )
        return f

    if only_jp == "streams":
        # one thunk list per (jp, hl) stream, for custom interleaves
        streams = {}
        for jp in range(NPAN):
            nch = 4 * jp + 4
            for hl in range(2):
                streams[(jp, hl)] = (
                    [chunk(hl, jp, i) for i in range(nch)]
                    + [finalize(hl, jp)]
                )
        return streams
    jps = range(NPAN) if only_jp is None else [only_jp]
    for jp in jps:
        nch = 4 * jp + 4
        for i in range(nch):
            for hl in range(2):
                thunks.append(chunk(hl, jp, i))
        for hl in range(2):
            thunks.append(finalize(hl, jp))
    return thunks


def _outproj_thunks(cx, jp):
    """Thunk list for the output projection of t-panel jp."""
    nc = cx.nc
    thunks = []

    def etile(et):
        def f():
            ps = cx.psO.tile([P, PAN], F32, tag="psO")
            for dc in range(HPC):
                nc.tensor.matmul(
                    ps,
                    lhsT=cx.wo_sb[:, dc, ds(et * P, P)],
                    rhs=cx.yt[dc // 2][:, dc % 2, ds(jp * PAN, PAN)],
                    start=(dc == 0),
                    stop=(dc == HPC - 1),
                )
            ob = cx.opool.tile([P, PAN], F32, tag="ob")
            nc.vector.tensor_copy(ob, ps)
            # alternate store queues: halves the drain at the kernel tail
            eng = nc.gpsimd if et % 2 == 0 else nc.scalar
            eng.dma_start(
                cx.out[ds(et * P, P), ds(jp * PAN, PAN)], ob
            )
        return f

    for et in range(E // P):
        thunks.append(etile(et))
    return thunks


def make_tables():
    j = np.arange(0, D, 2, dtype=np.float64) / D
    inv_freq = 1.0 / (10000.0 ** j)
    t = np.arange(T, dtype=np.float64)
    fr = np.outer(t, inv_freq)                            # [T, 64]
    c2 = np.cos(2.0 * fr).T                               # [64, T]
    s2 = np.sin(2.0 * fr).T
    cos2 = np.concatenate([c2, c2], axis=0).astype(np.float16)
    sin2 = np.concatenate([s2, -s2], axis=0).astype(np.float16)
    return cos2, sin2


def make_mask():
    s = np.arange(P)[:, None]
    c = np.arange(P)[None, :]
    return np.where(s <= c, 1.0, 0.0).astype(np.float16)


def _swizzle_xT(xt):
    # [E, T] -> [P, NXB, EO, XB]: per partition, each (xb) slice is one
    # contiguous 16KB run
    return np.ascontiguousarray(
        xt.reshape(EO, P, NXB, XB).transpose(1, 2, 0, 3)
    )


def _swizzle_wT(wt):
    # [E, 1024] -> [P, NGRP, EO, 256]
    return np.ascontiguousarray(
        wt.reshape(EO, P, NGRP, 2 * D).transpose(1, 2, 0, 3)
    )


def _swizzle_woT(wot):
    # [1024, E] -> [P, HPC, E]
    return np.ascontiguousarray(wot.reshape(HPC, P, E).transpose(1, 0, 2))


def make_in_maps(x, wq, wk, wv, wo):
    cos2, sin2 = make_tables()
    mask = make_mask()
    in_maps = []
    for c in range(8):
        b, hh = c // 2, c % 2
        rows = slice(hh * HPC * D, (hh + 1) * HPC * D)
        in_maps.append({
            "xP": _swizzle_xT(x[b].T.astype(np.float16)),
            "wqP": _swizzle_wT(wq[rows].T.astype(np.float16)),
            "wkP": _swizzle_wT(wk[rows].T.astype(np.float16)),
            "wvP": _swizzle_wT(wv[rows].T.astype(np.float16)),
            "woP": _swizzle_woT(wo[:, rows].T.astype(np.float16)),
            "cos2": cos2,
            "sin2": sin2,
            "mask": mask,
        })
    return in_maps


_PROGRAM_CACHE = {}


def get_program():
    if "nc" not in _PROGRAM_CACHE:
        _PROGRAM_CACHE["nc"] = build_program()
    return _PROGRAM_CACHE["nc"]


def kernel(x, wq, wk, wv, wo, _results_hook=None):
    x = np.asarray(x, dtype=np.float32)
    wq = np.asarray(wq, dtype=np.float32)
    wk = np.asarray(wk, dtype=np.float32)
    wv = np.asarray(wv, dtype=np.float32)
    wo = np.asarray(wo, dtype=np.float32)

    nc = get_program()
    in_maps = make_in_maps(x, wq, wk, wv, wo)
    res = run_bass_kernel_spmd(nc, in_maps, list(range(8)))
    if _results_hook is not None:
        _results_hook(res)
    outs = [r["out"] for r in res.results]
    full = np.empty((B, T, E), dtype=np.float32)
    for b in range(B):
        full[b] = (outs[2 * b] + outs[2 * b + 1]).T
    return full
